# revision 47
# baseline (speedup 1.0000x reference)
"""Trainium2 Bass kernel for nn_DINONewVq (VQ codebook lookup + softmax probs).

Strategy (per core; data-parallel over tokens, codebook replicated):
  - bf16 matmul (4x faster than f32 on PE) computes 2*S_approx for all
    (token, code) pairs; softmax probs come from exp(2*psum) directly
    (row constants cancel; bf16 error ~1e-5 rel, well within tolerance).
  - top-8 candidates per token selected by scanning E (max/max_index).
  - exact fix-up: for the 8 candidates only, recompute the reference's
    f32-quantized distance d = fl(fl(zz + cc) - 2*S) with exact f32 dot
    products, then pick the lowest-index minimum -- reproducing
    jnp.argmin's value-and-tie behaviour bitwise.
  - z_q_out = fl(z + fl(codebook[idx] - z)) (straight-through arithmetic,
    bitwise-identical to the reference), q_loss partials on device.
"""

import sys

sys.path.insert(0, "/opt/trn_rl_repo")

import numpy as np

import concourse.bacc as bacc
import concourse.bass as bass
import concourse.mybir as mybir
import concourse.tile as tile
from concourse.bass_utils import run_bass_kernel_spmd

F32 = mybir.dt.float32
BF16 = mybir.dt.bfloat16
U32 = mybir.dt.uint32

B, D, H, W = 32, 64, 32, 32
K = 4096
NCORES = 8
NB = B // NCORES          # batches per core
NLOC = NB * H * W         # tokens per core = 4096
NT = NLOC // 128          # token tiles per core = 32
NG = 4                    # tiles per fix-up group
NA = 6                    # candidates kept per token for the exact re-rank
AF = mybir.ActivationFunctionType
OP = mybir.AluOpType

_CACHE = {}


def build_nc():
    nc = bacc.Bacc("TRN2")

    z_in = nc.declare_dram_parameter("z", [NB, D, H * W], F32, isOutput=False)
    cb_in = nc.declare_dram_parameter("codebook", [K, D], F32, isOutput=False)
    ident_in = nc.declare_dram_parameter("ident", [128, 128], F32, isOutput=False)
    probs_out = nc.declare_dram_parameter("probs", [NLOC, K], F32, isOutput=True)
    zq_out = nc.declare_dram_parameter("zq", [NLOC, D], F32, isOutput=True)
    loss_out = nc.declare_dram_parameter("lossp", [128, 1], F32, isOutput=True)

    with tile.TileContext(nc) as tc:
        with (
            tc.tile_pool(name="const", bufs=1) as constp,
            tc.tile_pool(name="cbn", bufs=3) as cbnp,
            tc.tile_pool(name="zt", bufs=4) as ztp,
            tc.tile_pool(name="zbf", bufs=NT) as zbfp,
            tc.tile_pool(name="big", bufs=2) as bigp,
            tc.tile_pool(name="small", bufs=4) as smallp,
            tc.tile_pool(name="cand", bufs=3) as candp,
            tc.tile_pool(name="cscr", bufs=1) as scratchp,
            tc.tile_pool(name="fin", bufs=1) as finp,
            tc.tile_pool(name="ptr", bufs=2, space="PSUM") as ptrp,
            tc.tile_pool(name="ps", bufs=3, space="PSUM") as psp,
        ):
            ident = constp.tile([128, 128], F32, tag="ident")
            nc.sync.dma_start(out=ident[:], in_=ident_in[:, :])

            zN_all = finp.tile([128, NT * 64], F32, tag="zN_all")
            idxall = finp.tile([128, NT], U32, tag="idxall")
            zzall = finp.tile([128, NT], F32, tag="zzall")
            i8all = finp.tile([128, NT * 8], U32, tag="i8all")
            zqall = finp.tile([128, NT * 64], F32, tag="zqall")

            # cbT2bf = bf16(2 * codebook^T)  (64, K)
            cbT2bf = constp.tile([64, K], BF16, tag="cbT2bf")
            for kc in range(K // 128):
                cbn = cbnp.tile([128, 64], F32, tag="cbn")
                nc.sync.dma_start(out=cbn[:],
                                  in_=cb_in[kc * 128:(kc + 1) * 128, :])
                pt = ptrp.tile([64, 128], F32, tag="ptr")
                nc.tensor.transpose(pt[:], cbn[:], ident[:, :])
                nc.scalar.mul(cbT2bf[:, kc * 128:(kc + 1) * 128], pt[:], 2.0)

            # prepass: z tiles, bf16 copies, token-major z, row norms
            ztbfs = []
            for t in range(NT):
                b, j = divmod(t, H * W // 128)
                zT = ztp.tile([64, 128], F32, tag="zT")
                nc.sync.dma_start(out=zT[:], in_=z_in[b, :, j * 128:(j + 1) * 128])
                zTbf = zbfp.tile([64, 128], BF16, tag="zTbf")
                ztbfs.append(zTbf)
                nc.vector.tensor_copy(out=zTbf[:], in_=zT[:])
                pzN = ptrp.tile([128, 64], F32, tag="ptr")
                nc.tensor.transpose(pzN[:], zT[:], ident[0:64, 0:64])
                zNs = zN_all[:, t * 64:(t + 1) * 64]
                nc.vector.tensor_copy(out=zNs, in_=pzN[:])
                sqz = smallp.tile([128, 64], F32, tag="sqz")
                nc.vector.tensor_tensor(out=sqz[:], in0=zNs, in1=zNs, op=OP.mult)
                nc.vector.tensor_reduce(out=zzall[:, t:t + 1], in_=sqz[:],
                                        axis=mybir.AxisListType.X, op=OP.add)

            cands = []

            def fixup_group(g):
                """Exact f32 re-ranking of the top-8 candidates for 8 tiles."""
                t0 = g * NG
                cslice = slice(t0 * 8, (t0 + NG) * 8)       # (128, 64) cols
                cand = cands[g]                              # (128, NG*NA*64)
                c4 = cand[:].rearrange("p (t a b) -> p t a b", a=NA, b=64)
                sq = scratchp.tile([128, NG * NA * 64], F32, tag="cscratch")
                nc.vector.tensor_tensor(out=sq[:], in0=cand[:], in1=cand[:],
                                        op=OP.mult)
                cc8 = smallp.tile([128, NG * NA], F32, tag="cc8")
                cc8v = cc8[:].rearrange("p (t a) -> p t a", a=NA)
                nc.vector.tensor_reduce(
                    out=cc8v, in_=sq[:].rearrange("p (t a b) -> p t a b",
                                                  a=NA, b=64),
                    axis=mybir.AxisListType.X, op=OP.add)
                # dot products z . c for the 8 candidates of each token
                znb = zN_all[:, t0 * 64:(t0 + NG) * 64] \
                    .rearrange("p (t d) -> p t d", d=64) \
                    .unsqueeze(2).broadcast_to((128, NG, NA, 64))
                prod = scratchp.tile([128, NG * NA * 64], F32, tag="cscratch")
                prod4 = prod[:].rearrange("p (t a b) -> p t a b", a=NA, b=64)
                nc.vector.tensor_tensor(out=prod4, in0=c4, in1=znb, op=OP.mult)
                s8 = smallp.tile([128, NG * NA], F32, tag="s8")
                nc.vector.tensor_reduce(
                    out=s8[:].rearrange("p (t a) -> p t a", a=NA), in_=prod4,
                    axis=mybir.AxisListType.X, op=OP.add)
                # d = fl(fl(zz + cc) - 2S), reference rounding order
                zzb = zzall[:, t0:t0 + NG].unsqueeze(2) \
                    .broadcast_to((128, NG, NA))
                u8 = smallp.tile([128, NG * NA], F32, tag="u8")
                nc.vector.tensor_tensor(
                    out=u8[:].rearrange("p (t a) -> p t a", a=NA),
                    in0=cc8v, in1=zzb, op=OP.add)
                nc.vector.tensor_scalar(out=s8[:], in0=s8[:], scalar1=2.0,
                                        scalar2=0.0, op0=OP.mult)
                d8 = smallp.tile([128, NG * NA], F32, tag="d8")
                nc.vector.tensor_tensor(out=d8[:], in0=u8[:], in1=s8[:],
                                        op=OP.subtract)
                # lowest-index argmin among the 8 (jnp.argmin tie behaviour)
                mind = smallp.tile([128, NG], F32, tag="mind")
                nc.vector.tensor_reduce(
                    out=mind[:], in_=d8[:].rearrange("p (t a) -> p t a", a=NA),
                    axis=mybir.AxisListType.X, op=OP.min)
                mindb = mind[:].unsqueeze(2).broadcast_to((128, NG, NA))
                mask = smallp.tile([128, NG * NA], F32, tag="mask")
                nc.vector.tensor_tensor(
                    out=mask[:].rearrange("p (t a) -> p t a", a=NA),
                    in0=d8[:].rearrange("p (t a) -> p t a", a=NA), in1=mindb,
                    op=OP.is_equal)
                i8f = smallp.tile([128, NG * NA], F32, tag="i8f")
                nc.vector.tensor_copy(out=i8f[:].rearrange("p (t a) -> p t a", a=NA),
                                      in_=i8all[:, cslice].rearrange("p (t a) -> p t a", a=8)[:, :, 0:NA])
                # idxsel = i8f + 1e9*(1-mask)
                nc.vector.tensor_scalar(out=i8f[:], in0=i8f[:], scalar1=8192.0,
                                        scalar2=0.0, op0=OP.add)
                nc.vector.scalar_tensor_tensor(
                    out=i8f[:], in0=mask[:], scalar=-8192.0, in1=i8f[:],
                    op0=OP.mult, op1=OP.add)
                winf = smallp.tile([128, NG], F32, tag="winf")
                nc.vector.tensor_reduce(
                    out=winf[:], in_=i8f[:].rearrange("p (t a) -> p t a", a=NA),
                    axis=mybir.AxisListType.X, op=OP.min)
                nc.vector.tensor_copy(out=idxall[:, t0:t0 + NG], in_=winf[:])
                for tt in range(t0, t0 + NG):
                    nc.gpsimd.indirect_dma_start(
                        out=zqall[:, tt * 64:(tt + 1) * 64], out_offset=None,
                        in_=cb_in[:, :],
                        in_offset=bass.IndirectOffsetOnAxis(
                            ap=idxall[:, tt:tt + 1], axis=0))

            for t in range(NT):
                g, gt = divmod(t, NG)
                if gt == 0:
                    cand = candp.tile([128, NG * NA * 64], F32, tag="cand")
                    cands.append(cand)
                zTbf = ztbfs[t]

                E = bigp.tile([128, K], F32, tag="E")
                P = bigp.tile([128, K], F32, tag="P")
                sumparts = smallp.tile([128, 4], F32, tag="sumparts")

                for c in range(4):
                    ps = psp.tile([128, 1024], F32, tag="ps2s")
                    nc.tensor.matmul(out=ps[:, 0:512], lhsT=zTbf[:],
                                     rhs=cbT2bf[:, c * 1024:c * 1024 + 512],
                                     start=True, stop=True)
                    nc.tensor.matmul(out=ps[:, 512:1024], lhsT=zTbf[:],
                                     rhs=cbT2bf[:, c * 1024 + 512:(c + 1) * 1024],
                                     start=True, stop=True)
                    # E = exp(4*S_approx); row constants cancel in the softmax
                    nc.scalar.activation(
                        out=E[:, c * 1024:(c + 1) * 1024], in_=ps[:],
                        func=AF.Exp, scale=2.0,
                        accum_out=sumparts[:, c:c + 1])

                rowsum = smallp.tile([128, 1], F32, tag="rowsum")
                nc.vector.tensor_reduce(out=rowsum[:], in_=sumparts[:],
                                        axis=mybir.AxisListType.X, op=OP.add)
                recip = smallp.tile([128, 1], F32, tag="recip")
                nc.vector.reciprocal(out=recip[:], in_=rowsum[:])
                nc.scalar.activation(out=P[:], in_=E[:], func=AF.Copy,
                                     scale=recip[:])

                m8 = smallp.tile([128, 8], F32, tag="m8")
                nc.vector.max(out=m8[:], in_=E[:])
                i8 = smallp.tile([128, 8], U32, tag="i8")
                nc.vector.max_index(out=i8[:], in_max=m8[:], in_values=E[:])
                nc.vector.tensor_copy(out=i8all[:, t * 8:(t + 1) * 8],
                                      in_=i8[:])
                # gather the 8 candidate codebook rows for the exact re-rank
                # (hardware indirect DMA honours one offset per partition)
                for jj in range(NA):
                    nc.gpsimd.indirect_dma_start(
                        out=cand[:, (gt * NA + jj) * 64:(gt * NA + jj + 1) * 64],
                        out_offset=None, in_=cb_in[:, :],
                        in_offset=bass.IndirectOffsetOnAxis(
                            ap=i8[:, jj:jj + 1], axis=0))

                nc.sync.dma_start(out=probs_out[t * 128:(t + 1) * 128, :],
                                  in_=P[:])
                if gt == 0 and g >= 2:
                    fixup_group(g - 2)
                if gt == NG // 2 and g == NT // NG - 1:
                    fixup_group(g - 1)
            fixup_group(NT // NG - 1)

            # ---- final: straight-through output, loss ----
            diff = finp.tile([128, NT * 64], F32, tag="diff")
            nc.vector.tensor_tensor(out=diff[:], in0=zqall[:], in1=zN_all[:],
                                    op=OP.subtract)
            st = finp.tile([128, NT * 64], F32, tag="st")
            nc.vector.tensor_tensor(out=st[:], in0=zN_all[:], in1=diff[:],
                                    op=OP.add)
            # reuse zqall (dead) for squared residuals
            nc.vector.tensor_tensor(out=zqall[:], in0=diff[:], in1=diff[:],
                                    op=OP.mult)
            lossp = finp.tile([128, 1], F32, tag="lossp")
            nc.vector.tensor_reduce(out=lossp[:], in_=zqall[:],
                                    axis=mybir.AxisListType.X, op=OP.add)
            nc.sync.dma_start(
                out=zq_out[:, :].rearrange("(t p) d -> p t d", p=128),
                in_=st[:].rearrange("p (t d) -> p t d", t=NT))
            nc.sync.dma_start(out=loss_out[:, :], in_=lossp[:])

    nc.compile()
    return nc


def _get_nc():
    if "nc" not in _CACHE:
        _CACHE["nc"] = build_nc()
    return _CACHE["nc"]


def _install_ntff_hook():
    """The agent image's antenv lacks axon_hooks; recreate it so
    run_bass_kernel_spmd(trace=True) can capture NTFF profiles."""
    import types

    try:
        import antenv.axon_hooks  # noqa: F401
        return
    except ImportError:
        pass
    m = types.ModuleType("antenv.axon_hooks")
    m._hook = None
    m.set_axon_ntff_profile_hook = lambda h: setattr(m, "_hook", h)
    m.get_axon_ntff_profile_hook = lambda: m._hook
    sys.modules["antenv.axon_hooks"] = m
    import antenv

    antenv.axon_hooks = m
    from trn_agent_boot.trn_boot import _ntff_profile_via_ctypes

    m._hook = _ntff_profile_via_ctypes("/opt/axon/libaxon_pjrt.so")


def kernel(z, codebook, _trace=False):
    if _trace:
        _install_ntff_hook()
    nc = _get_nc()
    z = np.ascontiguousarray(z, dtype=np.float32)
    codebook = np.ascontiguousarray(codebook, dtype=np.float32)
    ident = np.eye(128, dtype=np.float32)

    in_maps = [
        {
            "z": np.ascontiguousarray(
                z[i * NB:(i + 1) * NB].reshape(NB, D, H * W)),
            "codebook": codebook,
            "ident": ident,
        }
        for i in range(NCORES)
    ]
    r = run_bass_kernel_spmd(nc, in_maps, core_ids=list(range(NCORES)),
                             trace=_trace)
    results = r.results

    probs = np.concatenate([results[i]["probs"] for i in range(NCORES)], axis=0)
    zq_flat = np.concatenate([results[i]["zq"] for i in range(NCORES)], axis=0)
    zq = np.transpose(zq_flat.reshape(B, H, W, D), (0, 3, 1, 2))
    total = np.sum([results[i]["lossp"].astype(np.float64).sum()
                    for i in range(NCORES)])
    q_loss = np.float32(1.25 * total / (B * H * W * D))

    if _trace:
        return (zq, q_loss, probs), r
    return zq, q_loss, probs


# revision 48
# speedup vs baseline: 1.0322x; 1.0322x over previous
"""Trainium2 Bass kernel for nn_DINONewVq (VQ codebook lookup + softmax probs).

Strategy (per core; data-parallel over tokens, codebook replicated):
  - bf16 matmul (4x faster than f32 on PE) computes 2*S_approx for all
    (token, code) pairs; softmax probs come from exp(2*psum) directly
    (row constants cancel; bf16 error ~1e-5 rel, well within tolerance).
  - top-8 candidates per token selected by scanning E (max/max_index).
  - exact fix-up: for the 8 candidates only, recompute the reference's
    f32-quantized distance d = fl(fl(zz + cc) - 2*S) with exact f32 dot
    products, then pick the lowest-index minimum -- reproducing
    jnp.argmin's value-and-tie behaviour bitwise.
  - z_q_out = fl(z + fl(codebook[idx] - z)) (straight-through arithmetic,
    bitwise-identical to the reference), q_loss partials on device.
"""

import sys

sys.path.insert(0, "/opt/trn_rl_repo")

import numpy as np

import concourse.bacc as bacc
import concourse.bass as bass
import concourse.mybir as mybir
import concourse.tile as tile
from concourse.bass_utils import run_bass_kernel_spmd

F32 = mybir.dt.float32
BF16 = mybir.dt.bfloat16
U32 = mybir.dt.uint32

B, D, H, W = 32, 64, 32, 32
K = 4096
NCORES = 8
NB = B // NCORES          # batches per core
NLOC = NB * H * W         # tokens per core = 4096
NT = NLOC // 128          # token tiles per core = 32
NG = 4                    # tiles per fix-up group
NA = 6                    # candidates kept per token for the exact re-rank
AF = mybir.ActivationFunctionType
OP = mybir.AluOpType

_CACHE = {}


def build_nc():
    nc = bacc.Bacc("TRN2")

    z_in = nc.declare_dram_parameter("z", [NB, D, H * W], F32, isOutput=False)
    cb_in = nc.declare_dram_parameter("codebook", [K, D], F32, isOutput=False)
    ident_in = nc.declare_dram_parameter("ident", [128, 128], F32, isOutput=False)
    probs_out = nc.declare_dram_parameter("probs", [NLOC, K], F32, isOutput=True)
    zq_out = nc.declare_dram_parameter("zq", [NLOC, D], F32, isOutput=True)
    loss_out = nc.declare_dram_parameter("lossp", [128, 1], F32, isOutput=True)

    with tile.TileContext(nc) as tc:
        with (
            tc.tile_pool(name="const", bufs=1) as constp,
            tc.tile_pool(name="cbn", bufs=3) as cbnp,
            tc.tile_pool(name="zt", bufs=4) as ztp,
            tc.tile_pool(name="big", bufs=2) as bigp,
            tc.tile_pool(name="small", bufs=4) as smallp,
            tc.tile_pool(name="cand", bufs=3) as candp,
            tc.tile_pool(name="cscr", bufs=1) as scratchp,
            tc.tile_pool(name="fin", bufs=1) as finp,
            tc.tile_pool(name="ptr", bufs=2, space="PSUM") as ptrp,
            tc.tile_pool(name="ps", bufs=3, space="PSUM") as psp,
        ):
            ident = constp.tile([128, 128], F32, tag="ident")
            nc.sync.dma_start(out=ident[:], in_=ident_in[:, :])

            # cbT2bf = bf16(2 * codebook^T)  (64, K)
            cbT2bf = constp.tile([64, K], BF16, tag="cbT2bf")
            for kc in range(K // 128):
                cbn = cbnp.tile([128, 64], F32, tag="cbn")
                nc.sync.dma_start(out=cbn[:],
                                  in_=cb_in[kc * 128:(kc + 1) * 128, :])
                pt = ptrp.tile([64, 128], F32, tag="ptr")
                nc.tensor.transpose(pt[:], cbn[:], ident[:, :])
                nc.scalar.mul(cbT2bf[:, kc * 128:(kc + 1) * 128], pt[:], 2.0)

            zN_all = finp.tile([128, NT * 64], F32, tag="zN_all")
            idxall = finp.tile([128, NT], U32, tag="idxall")
            zzall = finp.tile([128, NT], F32, tag="zzall")
            i8all = finp.tile([128, NT * 8], U32, tag="i8all")
            zqall = finp.tile([128, NT * 64], F32, tag="zqall")

            # prepass: z tiles, bf16 copies, token-major z, row norms
            ztbfs = []
            for t in range(NT):
                b, j = divmod(t, H * W // 128)
                zT = ztp.tile([64, 128], F32, tag="zT")
                nc.sync.dma_start(out=zT[:], in_=z_in[b, :, j * 128:(j + 1) * 128])
                zTbf = ztp.tile([64, 128], BF16, tag="zTbf")
                ztbfs.append(zTbf)
                nc.vector.tensor_copy(out=zTbf[:], in_=zT[:])
                pzN = ptrp.tile([128, 64], F32, tag="ptr")
                nc.tensor.transpose(pzN[:], zT[:], ident[0:64, 0:64])
                zNs = zN_all[:, t * 64:(t + 1) * 64]
                nc.vector.tensor_copy(out=zNs, in_=pzN[:])
                sqz = smallp.tile([128, 64], F32, tag="sqz")
                nc.vector.tensor_tensor(out=sqz[:], in0=zNs, in1=zNs, op=OP.mult)
                nc.vector.tensor_reduce(out=zzall[:, t:t + 1], in_=sqz[:],
                                        axis=mybir.AxisListType.X, op=OP.add)

            cands = []

            def fixup_group(g):
                """Exact f32 re-ranking of the top-8 candidates for 8 tiles."""
                t0 = g * NG
                cslice = slice(t0 * 8, (t0 + NG) * 8)       # (128, 64) cols
                cand = cands[g]                              # (128, NG*NA*64)
                c4 = cand[:].rearrange("p (t a b) -> p t a b", a=NA, b=64)
                sq = scratchp.tile([128, NG * NA * 64], F32, tag="cscratch")
                nc.vector.tensor_tensor(out=sq[:], in0=cand[:], in1=cand[:],
                                        op=OP.mult)
                cc8 = smallp.tile([128, NG * NA], F32, tag="cc8")
                cc8v = cc8[:].rearrange("p (t a) -> p t a", a=NA)
                nc.vector.tensor_reduce(
                    out=cc8v, in_=sq[:].rearrange("p (t a b) -> p t a b",
                                                  a=NA, b=64),
                    axis=mybir.AxisListType.X, op=OP.add)
                # dot products z . c for the 8 candidates of each token
                znb = zN_all[:, t0 * 64:(t0 + NG) * 64] \
                    .rearrange("p (t d) -> p t d", d=64) \
                    .unsqueeze(2).broadcast_to((128, NG, NA, 64))
                prod = scratchp.tile([128, NG * NA * 64], F32, tag="cscratch")
                prod4 = prod[:].rearrange("p (t a b) -> p t a b", a=NA, b=64)
                nc.vector.tensor_tensor(out=prod4, in0=c4, in1=znb, op=OP.mult)
                s8 = smallp.tile([128, NG * NA], F32, tag="s8")
                nc.vector.tensor_reduce(
                    out=s8[:].rearrange("p (t a) -> p t a", a=NA), in_=prod4,
                    axis=mybir.AxisListType.X, op=OP.add)
                # d = fl(fl(zz + cc) - 2S), reference rounding order
                zzb = zzall[:, t0:t0 + NG].unsqueeze(2) \
                    .broadcast_to((128, NG, NA))
                u8 = smallp.tile([128, NG * NA], F32, tag="u8")
                nc.vector.tensor_tensor(
                    out=u8[:].rearrange("p (t a) -> p t a", a=NA),
                    in0=cc8v, in1=zzb, op=OP.add)
                nc.vector.tensor_scalar(out=s8[:], in0=s8[:], scalar1=2.0,
                                        scalar2=0.0, op0=OP.mult)
                d8 = smallp.tile([128, NG * NA], F32, tag="d8")
                nc.vector.tensor_tensor(out=d8[:], in0=u8[:], in1=s8[:],
                                        op=OP.subtract)
                # lowest-index argmin among the 8 (jnp.argmin tie behaviour)
                mind = smallp.tile([128, NG], F32, tag="mind")
                nc.vector.tensor_reduce(
                    out=mind[:], in_=d8[:].rearrange("p (t a) -> p t a", a=NA),
                    axis=mybir.AxisListType.X, op=OP.min)
                mindb = mind[:].unsqueeze(2).broadcast_to((128, NG, NA))
                mask = smallp.tile([128, NG * NA], F32, tag="mask")
                nc.vector.tensor_tensor(
                    out=mask[:].rearrange("p (t a) -> p t a", a=NA),
                    in0=d8[:].rearrange("p (t a) -> p t a", a=NA), in1=mindb,
                    op=OP.is_equal)
                i8f = smallp.tile([128, NG * NA], F32, tag="i8f")
                nc.vector.tensor_copy(out=i8f[:].rearrange("p (t a) -> p t a", a=NA),
                                      in_=i8all[:, cslice].rearrange("p (t a) -> p t a", a=8)[:, :, 0:NA])
                # idxsel = i8f + 1e9*(1-mask)
                nc.vector.tensor_scalar(out=i8f[:], in0=i8f[:], scalar1=8192.0,
                                        scalar2=0.0, op0=OP.add)
                nc.vector.scalar_tensor_tensor(
                    out=i8f[:], in0=mask[:], scalar=-8192.0, in1=i8f[:],
                    op0=OP.mult, op1=OP.add)
                winf = smallp.tile([128, NG], F32, tag="winf")
                nc.vector.tensor_reduce(
                    out=winf[:], in_=i8f[:].rearrange("p (t a) -> p t a", a=NA),
                    axis=mybir.AxisListType.X, op=OP.min)
                nc.vector.tensor_copy(out=idxall[:, t0:t0 + NG], in_=winf[:])
                for tt in range(t0, t0 + NG):
                    nc.gpsimd.indirect_dma_start(
                        out=zqall[:, tt * 64:(tt + 1) * 64], out_offset=None,
                        in_=cb_in[:, :],
                        in_offset=bass.IndirectOffsetOnAxis(
                            ap=idxall[:, tt:tt + 1], axis=0))

            for t in range(NT):
                g, gt = divmod(t, NG)
                if gt == 0:
                    cand = candp.tile([128, NG * NA * 64], F32, tag="cand")
                    cands.append(cand)
                zTbf = ztbfs[t]

                E = bigp.tile([128, K], F32, tag="E")
                P = bigp.tile([128, K], F32, tag="P")
                sumparts = smallp.tile([128, 4], F32, tag="sumparts")

                for c in range(4):
                    ps = psp.tile([128, 1024], F32, tag="ps2s")
                    nc.tensor.matmul(out=ps[:, 0:512], lhsT=zTbf[:],
                                     rhs=cbT2bf[:, c * 1024:c * 1024 + 512],
                                     start=True, stop=True)
                    nc.tensor.matmul(out=ps[:, 512:1024], lhsT=zTbf[:],
                                     rhs=cbT2bf[:, c * 1024 + 512:(c + 1) * 1024],
                                     start=True, stop=True)
                    # E = exp(4*S_approx); row constants cancel in the softmax
                    nc.scalar.activation(
                        out=E[:, c * 1024:(c + 1) * 1024], in_=ps[:],
                        func=AF.Exp, scale=2.0,
                        accum_out=sumparts[:, c:c + 1])

                rowsum = smallp.tile([128, 1], F32, tag="rowsum")
                nc.vector.tensor_reduce(out=rowsum[:], in_=sumparts[:],
                                        axis=mybir.AxisListType.X, op=OP.add)
                recip = smallp.tile([128, 1], F32, tag="recip")
                nc.vector.reciprocal(out=recip[:], in_=rowsum[:])
                nc.scalar.activation(out=P[:], in_=E[:], func=AF.Copy,
                                     scale=recip[:])

                m8 = smallp.tile([128, 8], F32, tag="m8")
                nc.vector.max(out=m8[:], in_=E[:])
                i8 = smallp.tile([128, 8], U32, tag="i8")
                nc.vector.max_index(out=i8[:], in_max=m8[:], in_values=E[:])
                nc.vector.tensor_copy(out=i8all[:, t * 8:(t + 1) * 8],
                                      in_=i8[:])
                # gather the 8 candidate codebook rows for the exact re-rank
                # (hardware indirect DMA honours one offset per partition)
                for jj in range(NA):
                    nc.gpsimd.indirect_dma_start(
                        out=cand[:, (gt * NA + jj) * 64:(gt * NA + jj + 1) * 64],
                        out_offset=None, in_=cb_in[:, :],
                        in_offset=bass.IndirectOffsetOnAxis(
                            ap=i8[:, jj:jj + 1], axis=0))

                nc.sync.dma_start(out=probs_out[t * 128:(t + 1) * 128, :],
                                  in_=P[:])
                if gt == 0 and g >= 2:
                    fixup_group(g - 2)
                if gt == NG // 2 and g == NT // NG - 1:
                    fixup_group(g - 1)
            fixup_group(NT // NG - 1)

            # ---- final: straight-through output, loss ----
            diff = finp.tile([128, NT * 64], F32, tag="diff")
            nc.vector.tensor_tensor(out=diff[:], in0=zqall[:], in1=zN_all[:],
                                    op=OP.subtract)
            st = finp.tile([128, NT * 64], F32, tag="st")
            nc.vector.tensor_tensor(out=st[:], in0=zN_all[:], in1=diff[:],
                                    op=OP.add)
            # reuse zqall (dead) for squared residuals
            nc.vector.tensor_tensor(out=zqall[:], in0=diff[:], in1=diff[:],
                                    op=OP.mult)
            lossp = finp.tile([128, 1], F32, tag="lossp")
            nc.vector.tensor_reduce(out=lossp[:], in_=zqall[:],
                                    axis=mybir.AxisListType.X, op=OP.add)
            nc.sync.dma_start(
                out=zq_out[:, :].rearrange("(t p) d -> p t d", p=128),
                in_=st[:].rearrange("p (t d) -> p t d", t=NT))
            nc.sync.dma_start(out=loss_out[:, :], in_=lossp[:])

    nc.compile()
    return nc


def _get_nc():
    if "nc" not in _CACHE:
        _CACHE["nc"] = build_nc()
    return _CACHE["nc"]


def _install_ntff_hook():
    """The agent image's antenv lacks axon_hooks; recreate it so
    run_bass_kernel_spmd(trace=True) can capture NTFF profiles."""
    import types

    try:
        import antenv.axon_hooks  # noqa: F401
        return
    except ImportError:
        pass
    m = types.ModuleType("antenv.axon_hooks")
    m._hook = None
    m.set_axon_ntff_profile_hook = lambda h: setattr(m, "_hook", h)
    m.get_axon_ntff_profile_hook = lambda: m._hook
    sys.modules["antenv.axon_hooks"] = m
    import antenv

    antenv.axon_hooks = m
    from trn_agent_boot.trn_boot import _ntff_profile_via_ctypes

    m._hook = _ntff_profile_via_ctypes("/opt/axon/libaxon_pjrt.so")


def kernel(z, codebook, _trace=False):
    if _trace:
        _install_ntff_hook()
    nc = _get_nc()
    z = np.ascontiguousarray(z, dtype=np.float32)
    codebook = np.ascontiguousarray(codebook, dtype=np.float32)
    ident = np.eye(128, dtype=np.float32)

    in_maps = [
        {
            "z": np.ascontiguousarray(
                z[i * NB:(i + 1) * NB].reshape(NB, D, H * W)),
            "codebook": codebook,
            "ident": ident,
        }
        for i in range(NCORES)
    ]
    r = run_bass_kernel_spmd(nc, in_maps, core_ids=list(range(NCORES)),
                             trace=_trace)
    results = r.results

    probs = np.concatenate([results[i]["probs"] for i in range(NCORES)], axis=0)
    zq_flat = np.concatenate([results[i]["zq"] for i in range(NCORES)], axis=0)
    zq = np.transpose(zq_flat.reshape(B, H, W, D), (0, 3, 1, 2))
    total = np.sum([results[i]["lossp"].astype(np.float64).sum()
                    for i in range(NCORES)])
    q_loss = np.float32(1.25 * total / (B * H * W * D))

    if _trace:
        return (zq, q_loss, probs), r
    return zq, q_loss, probs


# revision 49
# speedup vs baseline: 1.0731x; 1.0397x over previous
"""Trainium2 Bass kernel for nn_DINONewVq (VQ codebook lookup + softmax probs).

Strategy (per core; data-parallel over tokens, codebook replicated):
  - bf16 matmul (4x faster than f32 on PE) computes 2*S_approx for all
    (token, code) pairs; softmax probs come from exp(2*psum) directly
    (row constants cancel; bf16 error ~1e-5 rel, well within tolerance).
  - top-8 candidates per token selected by scanning E (max/max_index).
  - exact fix-up: for the 8 candidates only, recompute the reference's
    f32-quantized distance d = fl(fl(zz + cc) - 2*S) with exact f32 dot
    products, then pick the lowest-index minimum -- reproducing
    jnp.argmin's value-and-tie behaviour bitwise.
  - z_q_out = fl(z + fl(codebook[idx] - z)) (straight-through arithmetic,
    bitwise-identical to the reference), q_loss partials on device.
"""

import sys

sys.path.insert(0, "/opt/trn_rl_repo")

import numpy as np

import concourse.bacc as bacc
import concourse.bass as bass
import concourse.mybir as mybir
import concourse.tile as tile
from concourse.bass_utils import run_bass_kernel_spmd

F32 = mybir.dt.float32
BF16 = mybir.dt.bfloat16
U32 = mybir.dt.uint32

B, D, H, W = 32, 64, 32, 32
K = 4096
NCORES = 8
NB = B // NCORES          # batches per core
NLOC = NB * H * W         # tokens per core = 4096
NT = NLOC // 128          # token tiles per core = 32
NG = 4                    # tiles per fix-up group
NA = 6                    # candidates kept per token for the exact re-rank
AF = mybir.ActivationFunctionType
OP = mybir.AluOpType

_CACHE = {}


def build_nc():
    nc = bacc.Bacc("TRN2")

    z_in = nc.declare_dram_parameter("z", [NB, D, H * W], F32, isOutput=False)
    cb_in = nc.declare_dram_parameter("codebook", [K, D], F32, isOutput=False)
    ident_in = nc.declare_dram_parameter("ident", [128, 128], F32, isOutput=False)
    probs_out = nc.declare_dram_parameter("probs", [NLOC, K], F32, isOutput=True)
    zq_out = nc.declare_dram_parameter("zq", [NLOC, D], F32, isOutput=True)
    loss_out = nc.declare_dram_parameter("lossp", [128, 1], F32, isOutput=True)

    with tile.TileContext(nc) as tc:
        with (
            tc.tile_pool(name="const", bufs=1) as constp,
            tc.tile_pool(name="cbn", bufs=3) as cbnp,
            tc.tile_pool(name="zt", bufs=4) as ztp,
            tc.tile_pool(name="big", bufs=2) as bigp,
            tc.tile_pool(name="small", bufs=4) as smallp,
            tc.tile_pool(name="cand", bufs=3) as candp,
            tc.tile_pool(name="cscr", bufs=1) as scratchp,
            tc.tile_pool(name="fin", bufs=1) as finp,
            tc.tile_pool(name="ptr", bufs=2, space="PSUM") as ptrp,
            tc.tile_pool(name="ps", bufs=3, space="PSUM") as psp,
        ):
            ident = constp.tile([128, 128], F32, tag="ident")
            nc.sync.dma_start(out=ident[:], in_=ident_in[:, :])

            # cbT2bf = bf16(2 * codebook^T)  (64, K)
            cbT2bf = constp.tile([64, K], BF16, tag="cbT2bf")
            for kc in range(K // 128):
                cbn = cbnp.tile([128, 64], F32, tag="cbn")
                nc.sync.dma_start(out=cbn[:],
                                  in_=cb_in[kc * 128:(kc + 1) * 128, :])
                pt = ptrp.tile([64, 128], F32, tag="ptr")
                nc.tensor.transpose(pt[:], cbn[:], ident[:, :])
                nc.scalar.mul(cbT2bf[:, kc * 128:(kc + 1) * 128], pt[:], 2.0)

            zN_all = finp.tile([128, NT * 64], F32, tag="zN_all")
            idxall = finp.tile([128, NT], U32, tag="idxall")
            zzall = finp.tile([128, NT], F32, tag="zzall")
            i8all = finp.tile([128, NT * 8], U32, tag="i8all")
            zqall = finp.tile([128, NT * 64], F32, tag="zqall")

            # prepass: z tiles, bf16 copies, token-major z, row norms
            ztbfs = []
            for t in range(NT):
                b, j = divmod(t, H * W // 128)
                zT = ztp.tile([64, 128], F32, tag="zT")
                nc.sync.dma_start(out=zT[:], in_=z_in[b, :, j * 128:(j + 1) * 128])
                zTbf = ztp.tile([64, 128], BF16, tag="zTbf")
                ztbfs.append(zTbf)
                nc.vector.tensor_copy(out=zTbf[:], in_=zT[:])
                pzN = ptrp.tile([128, 64], F32, tag="ptr")
                nc.tensor.transpose(pzN[:], zT[:], ident[0:64, 0:64])
                zNs = zN_all[:, t * 64:(t + 1) * 64]
                nc.vector.tensor_copy(out=zNs, in_=pzN[:])
                sqz = smallp.tile([128, 64], F32, tag="sqz")
                nc.scalar.square(sqz[:], zNs)
                nc.vector.tensor_reduce(out=zzall[:, t:t + 1], in_=sqz[:],
                                        axis=mybir.AxisListType.X, op=OP.add)

            cands = []

            def fixup_group(g):
                """Exact f32 re-ranking of the top-8 candidates for 8 tiles."""
                t0 = g * NG
                cslice = slice(t0 * 8, (t0 + NG) * 8)       # (128, 64) cols
                cand = cands[g]                              # (128, NG*NA*64)
                c4 = cand[:].rearrange("p (t a b) -> p t a b", a=NA, b=64)
                sq = scratchp.tile([128, NG * NA * 64], F32, tag="cscratch")
                nc.scalar.square(sq[:], cand[:])
                cc8 = smallp.tile([128, NG * NA], F32, tag="cc8")
                cc8v = cc8[:].rearrange("p (t a) -> p t a", a=NA)
                nc.vector.tensor_reduce(
                    out=cc8v, in_=sq[:].rearrange("p (t a b) -> p t a b",
                                                  a=NA, b=64),
                    axis=mybir.AxisListType.X, op=OP.add)
                # dot products z . c for the 8 candidates of each token
                znb = zN_all[:, t0 * 64:(t0 + NG) * 64] \
                    .rearrange("p (t d) -> p t d", d=64) \
                    .unsqueeze(2).broadcast_to((128, NG, NA, 64))
                prod = scratchp.tile([128, NG * NA * 64], F32, tag="cscratch")
                prod4 = prod[:].rearrange("p (t a b) -> p t a b", a=NA, b=64)
                nc.vector.tensor_tensor(out=prod4, in0=c4, in1=znb, op=OP.mult)
                s8 = smallp.tile([128, NG * NA], F32, tag="s8")
                nc.vector.tensor_reduce(
                    out=s8[:].rearrange("p (t a) -> p t a", a=NA), in_=prod4,
                    axis=mybir.AxisListType.X, op=OP.add)
                # d = fl(fl(zz + cc) - 2S), reference rounding order
                zzb = zzall[:, t0:t0 + NG].unsqueeze(2) \
                    .broadcast_to((128, NG, NA))
                u8 = smallp.tile([128, NG * NA], F32, tag="u8")
                nc.vector.tensor_tensor(
                    out=u8[:].rearrange("p (t a) -> p t a", a=NA),
                    in0=cc8v, in1=zzb, op=OP.add)
                nc.vector.tensor_scalar(out=s8[:], in0=s8[:], scalar1=2.0,
                                        scalar2=0.0, op0=OP.mult)
                d8 = smallp.tile([128, NG * NA], F32, tag="d8")
                nc.vector.tensor_tensor(out=d8[:], in0=u8[:], in1=s8[:],
                                        op=OP.subtract)
                # lowest-index argmin among the 8 (jnp.argmin tie behaviour)
                mind = smallp.tile([128, NG], F32, tag="mind")
                nc.vector.tensor_reduce(
                    out=mind[:], in_=d8[:].rearrange("p (t a) -> p t a", a=NA),
                    axis=mybir.AxisListType.X, op=OP.min)
                mindb = mind[:].unsqueeze(2).broadcast_to((128, NG, NA))
                mask = smallp.tile([128, NG * NA], F32, tag="mask")
                nc.vector.tensor_tensor(
                    out=mask[:].rearrange("p (t a) -> p t a", a=NA),
                    in0=d8[:].rearrange("p (t a) -> p t a", a=NA), in1=mindb,
                    op=OP.is_equal)
                i8f = smallp.tile([128, NG * NA], F32, tag="i8f")
                nc.vector.tensor_copy(out=i8f[:].rearrange("p (t a) -> p t a", a=NA),
                                      in_=i8all[:, cslice].rearrange("p (t a) -> p t a", a=8)[:, :, 0:NA])
                # idxsel = i8f + 1e9*(1-mask)
                nc.vector.tensor_scalar(out=i8f[:], in0=i8f[:], scalar1=8192.0,
                                        scalar2=0.0, op0=OP.add)
                nc.vector.scalar_tensor_tensor(
                    out=i8f[:], in0=mask[:], scalar=-8192.0, in1=i8f[:],
                    op0=OP.mult, op1=OP.add)
                winf = smallp.tile([128, NG], F32, tag="winf")
                nc.vector.tensor_reduce(
                    out=winf[:], in_=i8f[:].rearrange("p (t a) -> p t a", a=NA),
                    axis=mybir.AxisListType.X, op=OP.min)
                nc.vector.tensor_copy(out=idxall[:, t0:t0 + NG], in_=winf[:])
                for tt in range(t0, t0 + NG):
                    nc.gpsimd.indirect_dma_start(
                        out=zqall[:, tt * 64:(tt + 1) * 64], out_offset=None,
                        in_=cb_in[:, :],
                        in_offset=bass.IndirectOffsetOnAxis(
                            ap=idxall[:, tt:tt + 1], axis=0))

            for t in range(NT):
                g, gt = divmod(t, NG)
                if gt == 0:
                    cand = candp.tile([128, NG * NA * 64], F32, tag="cand")
                    cands.append(cand)
                zTbf = ztbfs[t]

                E = bigp.tile([128, K], F32, tag="E")
                P = bigp.tile([128, K], F32, tag="P")
                sumparts = smallp.tile([128, 4], F32, tag="sumparts")

                for c in range(4):
                    ps = psp.tile([128, 1024], F32, tag="ps2s")
                    nc.tensor.matmul(out=ps[:, 0:512], lhsT=zTbf[:],
                                     rhs=cbT2bf[:, c * 1024:c * 1024 + 512],
                                     start=True, stop=True)
                    nc.tensor.matmul(out=ps[:, 512:1024], lhsT=zTbf[:],
                                     rhs=cbT2bf[:, c * 1024 + 512:(c + 1) * 1024],
                                     start=True, stop=True)
                    # E = exp(4*S_approx); row constants cancel in the softmax
                    nc.scalar.activation(
                        out=E[:, c * 1024:(c + 1) * 1024], in_=ps[:],
                        func=AF.Exp, scale=2.0,
                        accum_out=sumparts[:, c:c + 1])

                rowsum = smallp.tile([128, 1], F32, tag="rowsum")
                nc.vector.tensor_reduce(out=rowsum[:], in_=sumparts[:],
                                        axis=mybir.AxisListType.X, op=OP.add)
                recip = smallp.tile([128, 1], F32, tag="recip")
                nc.vector.reciprocal(out=recip[:], in_=rowsum[:])
                nc.scalar.activation(out=P[:], in_=E[:], func=AF.Copy,
                                     scale=recip[:])

                m8 = smallp.tile([128, 8], F32, tag="m8")
                nc.vector.max(out=m8[:], in_=E[:])
                i8 = smallp.tile([128, 8], U32, tag="i8")
                nc.vector.max_index(out=i8[:], in_max=m8[:], in_values=E[:])
                nc.vector.tensor_copy(out=i8all[:, t * 8:(t + 1) * 8],
                                      in_=i8[:])
                # gather the 8 candidate codebook rows for the exact re-rank
                # (hardware indirect DMA honours one offset per partition)
                for jj in range(NA):
                    nc.gpsimd.indirect_dma_start(
                        out=cand[:, (gt * NA + jj) * 64:(gt * NA + jj + 1) * 64],
                        out_offset=None, in_=cb_in[:, :],
                        in_offset=bass.IndirectOffsetOnAxis(
                            ap=i8[:, jj:jj + 1], axis=0))

                nc.sync.dma_start(out=probs_out[t * 128:(t + 1) * 128, :],
                                  in_=P[:])
                if gt == 0 and g >= 2:
                    fixup_group(g - 2)
                if gt == NG // 2 and g == NT // NG - 1:
                    fixup_group(g - 1)
            fixup_group(NT // NG - 1)

            # ---- final: straight-through output, loss ----
            diff = finp.tile([128, NT * 64], F32, tag="diff")
            nc.vector.tensor_tensor(out=diff[:], in0=zqall[:], in1=zN_all[:],
                                    op=OP.subtract)
            st = finp.tile([128, NT * 64], F32, tag="st")
            nc.vector.tensor_tensor(out=st[:], in0=zN_all[:], in1=diff[:],
                                    op=OP.add)
            # reuse zqall (dead) for squared residuals
            nc.vector.tensor_tensor(out=zqall[:], in0=diff[:], in1=diff[:],
                                    op=OP.mult)
            lossp = finp.tile([128, 1], F32, tag="lossp")
            nc.vector.tensor_reduce(out=lossp[:], in_=zqall[:],
                                    axis=mybir.AxisListType.X, op=OP.add)
            nc.sync.dma_start(
                out=zq_out[:, :].rearrange("(t p) d -> p t d", p=128),
                in_=st[:].rearrange("p (t d) -> p t d", t=NT))
            nc.sync.dma_start(out=loss_out[:, :], in_=lossp[:])

    nc.compile()
    return nc


def _get_nc():
    if "nc" not in _CACHE:
        _CACHE["nc"] = build_nc()
    return _CACHE["nc"]


def _install_ntff_hook():
    """The agent image's antenv lacks axon_hooks; recreate it so
    run_bass_kernel_spmd(trace=True) can capture NTFF profiles."""
    import types

    try:
        import antenv.axon_hooks  # noqa: F401
        return
    except ImportError:
        pass
    m = types.ModuleType("antenv.axon_hooks")
    m._hook = None
    m.set_axon_ntff_profile_hook = lambda h: setattr(m, "_hook", h)
    m.get_axon_ntff_profile_hook = lambda: m._hook
    sys.modules["antenv.axon_hooks"] = m
    import antenv

    antenv.axon_hooks = m
    from trn_agent_boot.trn_boot import _ntff_profile_via_ctypes

    m._hook = _ntff_profile_via_ctypes("/opt/axon/libaxon_pjrt.so")


def kernel(z, codebook, _trace=False):
    if _trace:
        _install_ntff_hook()
    nc = _get_nc()
    z = np.ascontiguousarray(z, dtype=np.float32)
    codebook = np.ascontiguousarray(codebook, dtype=np.float32)
    ident = np.eye(128, dtype=np.float32)

    in_maps = [
        {
            "z": np.ascontiguousarray(
                z[i * NB:(i + 1) * NB].reshape(NB, D, H * W)),
            "codebook": codebook,
            "ident": ident,
        }
        for i in range(NCORES)
    ]
    r = run_bass_kernel_spmd(nc, in_maps, core_ids=list(range(NCORES)),
                             trace=_trace)
    results = r.results

    probs = np.concatenate([results[i]["probs"] for i in range(NCORES)], axis=0)
    zq_flat = np.concatenate([results[i]["zq"] for i in range(NCORES)], axis=0)
    zq = np.transpose(zq_flat.reshape(B, H, W, D), (0, 3, 1, 2))
    total = np.sum([results[i]["lossp"].astype(np.float64).sum()
                    for i in range(NCORES)])
    q_loss = np.float32(1.25 * total / (B * H * W * D))

    if _trace:
        return (zq, q_loss, probs), r
    return zq, q_loss, probs
